# revision 23
# baseline (speedup 1.0000x reference)
"""DeepWalk hierarchical-softmax loss on 8 Trainium2 NeuronCores.

loss[b] = sum_{l=1..18} softplus(-sign_l * dot(probs[(ctx_b+N) >> l], emb[center_b]))

Strategy (v3 — per-chunk pipeline, balanced engines):
  Elements are context-sorted and split 8192/core (64 tiles of 128).
  Sorted order makes consecutive elements share tree nodes at levels >= 3:
  a 128-element tile touches only ~214 distinct (level, node) rows across
  levels 3..18 (vs 16*128 naive).

  Levels 3..18 ("PE path"): per tile, the distinct sign-folded rows are a
  [128d x W] fp8 moving block; stationary is the tile's embeddings
  transposed [128d x 128b] fp16.  One matmul gives every element's dot
  against every candidate row (PSUM [128b x W]).  Four 512-wide matmuls
  (identity stationary x fp8 bias matrix) add 0 to each element's OWN
  (level, node) columns and -64 to the rest; softplus(dot - 64) ~= 0, so
  a plain per-tile row sum after softplus performs the per-element
  selection with no gather/select ops at all.

  softplus = Ln(Exp(x)+1) on the scalar engine; an explicit
  InstLoadActFuncSet pins the combined natural_log_exp table once so the
  per-chunk Exp/Ln interleave never reloads tables (the v2 kernel lost
  18 us to table thrash).  The row sum runs as: two folds on the (idle)
  GPSIMD engine, then one segmented pool_avg on the DVE; the avg's 1/(W/4)
  factor is undone by a per-tile weight vector at the end.

  Levels 1..2 ("dense path"): nearly every element has a distinct node, so
  the rows are host-gathered per element (contiguous fp16 streams) and the
  dots run on the DVE as one multiply + two folds + one segmented pool_avg
  per chunk; the 1/32 avg factor is undone by the Exp activation's scale.

  All six per-chunk input tensors are packed into two DMA slabs (one fp16,
  one fp8) so the stream is 2 triggers/chunk and arrives in issue order.
  Tiles are width-sorted per core so each 8-tile chunk's padded width
  tracks the envelope, not the global max; the last chunk (narrowest) also
  minimizes the pipeline tail.  All host work is layout only: sorting,
  index building, row gathering/sign folding, dtype casts.
"""

import numpy as np

N_NODES = 524288          # 2**19
D = 128
B_TOTAL = 65536
N_CORES = 8
BC = B_TOTAL // N_CORES   # 8192
T = BC // 128             # 64 tiles/core
CH = 8                    # tiles per chunk
NCH = T // CH             # 8 chunks
SLOT = 256                # psum slot stride (fp32 elems); 2 slots per bank
L_DENSE = (1, 2)
L_PE = tuple(range(3, 19))
MASK_OFF = -64.0          # softplus(dot + MASK_OFF) ~= 0 for |dot| <~ 40

_CACHE = {}


def _build_program(widths):
    """widths: per-chunk padded column counts (shared across cores)."""
    import os as _os
    import concourse.mybir as mybir
    import concourse.tile as tile
    import concourse.bacc as bacc
    from concourse.hw_specs import get_activation_tables

    f32 = mybir.dt.float32
    f16 = mybir.dt.float16
    f8 = mybir.dt.float8e4
    mult = mybir.AluOpType.mult
    add = mybir.AluOpType.add
    f_exp = mybir.ActivationFunctionType.Exp
    f_ln = mybir.ActivationFunctionType.Ln

    nc = bacc.Bacc("TRN2", target_bir_lowering=False, debug=False,
                   num_devices=int(_os.environ.get("KERNEL_NDEV", N_CORES)))

    tab_names = list(get_activation_tables(nc.m.arch).keys())
    LN_EXP_TABLE = tab_names.index("natural_log_exp_and_others")

    mv_cols = sum(CH * w for w in widths)          # mov cols (packed W)
    s8_cols = 2 * mv_cols                          # + mask cols (packed W)
    slabT = nc.dram_tensor("slabT", (128, NCH * CH * D), f16,
                           kind="ExternalInput").ap()
    slab16 = nc.dram_tensor("slab16", (128, NCH * 3 * CH * D), f16,
                            kind="ExternalInput").ap()
    slab8 = nc.dram_tensor("slab8", (128, s8_cols), f8,
                           kind="ExternalInput").ap()
    ident = nc.dram_tensor("ident", (128, 128), f8, kind="ExternalInput").ap()
    out = nc.dram_tensor("out", (128, T), f32, kind="ExternalOutput").ap()

    with tile.TileContext(nc) as tc:
        with tc.tile_pool(name="cst", bufs=1) as pcst, \
             tc.tile_pool(name="sT", bufs=6) as psT, \
             tc.tile_pool(name="s16", bufs=6) as ps16, \
             tc.tile_pool(name="s8", bufs=6) as ps8, \
             tc.tile_pool(name="sp", bufs=2) as psp, \
             tc.tile_pool(name="df", bufs=2) as pdf, \
             tc.tile_pool(name="dn", bufs=2) as pdn, \
             tc.tile_pool(name="acc", bufs=1) as pacc, \
             tc.tile_pool(name="ps", bufs=2, space="PSUM") as pps:

            # pin the combined exp+ln activation table once
            nc.scalar.add_instruction(mybir.InstLoadActFuncSet(
                name=nc.get_next_instruction_name(), ins=[], outs=[],
                act_func_set_id=LN_EXP_TABLE))

            t_id = pcst.tile([128, 128], f8, tag="ID")
            nc.gpsimd.dma_start(out=t_id[:], in_=ident)

            ydeep = pacc.tile([128, T], f32, tag="YD")    # pooled deep avgs
            y12 = pacc.tile([128, 2 * T], f32, tag="Y12")  # pooled dense avgs

            off8 = 0
            for c in range(NCH):
                W = widths[c]
                H, Q = W // 2, W // 4
                c16 = slice(c * 3 * CH * D, (c + 1) * 3 * CH * D)
                cT = slice(c * CH * D, (c + 1) * CH * D)
                tT = psT.tile([128, CH * D], f16, tag="ST")
                t16 = ps16.tile([128, 3 * CH * D], f16, tag="S16")
                t8 = ps8.tile([128, 2 * CH * W], f8, tag="S8")
                # dense-path (DVE backlog) operands stream first
                nc.sync.dma_start(out=t16[:], in_=slab16[:, c16])
                nc.sync.dma_start(out=tT[:], in_=slabT[:, cT])
                nc.sync.dma_start(
                    out=t8[:], in_=slab8[:, off8: off8 + 2 * CH * W])
                off8 += 2 * CH * W

                embT = tT[:, 0: CH * D]
                emb = t16[:, 0: CH * D]
                g1 = t16[:, CH * D: 2 * CH * D]
                g2 = t16[:, 2 * CH * D: 3 * CH * D]
                mov = t8[:, 0: CH * W]
                mask = t8[:, CH * W: 2 * CH * W]

                # ---- PE path: dots + bias for CH tiles into one psum ----
                # start=True clears the whole 2-slot PSUM bank, so only the
                # even-slot matmul per bank sets it; 512-wide identity
                # matmuls then add the per-element -64/0 bias per bank.
                big = pps.tile([128, CH * SLOT], f32, tag="PS")
                for i in range(0, CH, 2):
                    nc.tensor.matmul(
                        big[:, i * SLOT: i * SLOT + W],
                        lhsT=embT[:, i * 128: (i + 1) * 128],
                        rhs=mov[:, i * W: (i + 1) * W],
                        start=True, stop=False, skip_group_check=True)
                for i in range(1, CH, 2):
                    nc.tensor.matmul(
                        big[:, i * SLOT: i * SLOT + W],
                        lhsT=embT[:, i * 128: (i + 1) * 128],
                        rhs=mov[:, i * W: (i + 1) * W],
                        start=False, stop=False, skip_group_check=True)
                for i in range(CH):
                    nc.tensor.matmul(
                        big[:, i * SLOT: i * SLOT + W],
                        lhsT=t_id[:],
                        rhs=mask[:, i * W: (i + 1) * W],
                        start=False, stop=True, skip_group_check=True)

                # ---- softplus = Ln(Exp(x) + 1) ----
                # Exp runs in place in PSUM (elementwise streaming RMW), so
                # no fp32 intermediate ever touches SBUF; Ln then writes the
                # fp16 softplus straight to SBUF.
                ps3 = big[:].rearrange("p (s w) -> p s w", w=SLOT)[:, :, :W]
                nc.scalar.activation(out=ps3, in_=ps3, func=f_exp)
                t_sp = psp.tile([128, CH * W], f16, tag="SP")
                sp3 = t_sp[:].rearrange("p (s w) -> p s w", w=W)
                nc.scalar.activation(out=sp3, in_=ps3, func=f_ln, bias=1.0)

                # ---- deep row-sum: 2 folds on GPSIMD + pool_avg on DVE ----
                t_f1 = pdf.tile([128, CH * H], f16, tag="DF1")
                f13 = t_f1[:].rearrange("p (s w) -> p s w", w=H)
                nc.gpsimd.tensor_tensor(
                    out=f13, in0=sp3[:, :, 0:H], in1=sp3[:, :, H:W], op=add)
                t_f2 = pdf.tile([128, CH * Q], f16, tag="DF2")
                f23 = t_f2[:].rearrange("p (s w) -> p s w", w=Q)
                nc.gpsimd.tensor_tensor(
                    out=f23, in0=f13[:, :, 0:Q], in1=f13[:, :, Q:H], op=add)

                # ---- dense path: levels 1..2 on DVE ----
                prod = pdn.tile([128, 2 * CH * D], f16, tag="PR")
                nc.vector.tensor_tensor(
                    out=prod[:, 0: CH * D], in0=g1, in1=emb, op=mult)
                nc.vector.tensor_tensor(
                    out=prod[:, CH * D: 2 * CH * D], in0=g2, in1=emb, op=mult)
                p4 = prod[:].rearrange("p (l s d) -> p l s d", l=2, d=D)
                t_p1 = pdn.tile([128, 2 * CH * (D // 2)], f16, tag="PF1")
                p14 = t_p1[:].rearrange("p (l s d) -> p l s d", l=2, d=D // 2)
                nc.vector.tensor_tensor(
                    out=p14, in0=p4[:, :, :, 0: D // 2],
                    in1=p4[:, :, :, D // 2: D], op=add)
                t_p2 = pdn.tile([128, 2 * CH * (D // 4)], f16, tag="PF2")
                p24 = t_p2[:].rearrange("p (l s d) -> p l s d", l=2, d=D // 4)
                nc.vector.tensor_tensor(
                    out=p24, in0=p14[:, :, :, 0: D // 4],
                    in1=p14[:, :, :, D // 4: D // 2], op=add)
                y124 = y12[:].rearrange("p (l t) -> p l t", l=2)
                nc.vector.tensor_reduce(
                    out=y124[:, :, c * CH: (c + 1) * CH], in_=p24,
                    axis=mybir.AxisListType.X, op=add)
                nc.vector.tensor_reduce(
                    out=ydeep[:, c * CH: (c + 1) * CH], in_=f23,
                    axis=mybir.AxisListType.X, op=add)

            # ---- tail: softplus(dense dots) and final sum ----
            sp12 = pacc.tile([128, 2 * T], f32, tag="SP12")
            nc.scalar.activation(out=sp12[:], in_=y12[:], func=f_exp)
            nc.scalar.activation(out=sp12[:], in_=sp12[:], func=f_ln, bias=1.0)
            tot = pacc.tile([128, T], f32, tag="TOT")
            nc.vector.tensor_tensor(
                out=tot[:], in0=ydeep[:], in1=sp12[:, 0: T], op=add)
            nc.vector.tensor_tensor(
                out=tot[:], in0=tot[:], in1=sp12[:, T: 2 * T], op=add)
            nc.sync.dma_start(out=out, in_=tot[:])

    nc.compile()
    return nc


def _prep(center, context, embeddings, probs):
    """Host layout prep: sort, shard, build per-core tensors."""
    import concourse.mybir as mybir
    np_f8 = mybir.dt.np(mybir.dt.float8e4)

    perm = np.argsort(context, kind="stable")

    cores = []
    for c in range(N_CORES):
        shard = perm[c * BC: (c + 1) * BC]
        ctx = context[shard].astype(np.int64) + N_NODES
        cen = center[shard].astype(np.int64)

        # per-tile distinct rows for levels 3..18
        uniq_nodes = []   # per tile: concat distinct node ids (level-major)
        colidx = np.empty((T, 128, len(L_PE)), dtype=np.int64)
        W_t = np.empty(T, dtype=np.int64)
        for t in range(T):
            tc = ctx[t * 128: (t + 1) * 128]
            nodes_t = []
            base = 0
            for k, l in enumerate(L_PE):
                u, inv = np.unique(tc >> l, return_inverse=True)
                nodes_t.append(u)
                colidx[t, :, k] = base + inv
                base += len(u)
            uniq_nodes.append(np.concatenate(nodes_t))
            W_t[t] = base

        cores.append(dict(shard=shard, ctx=ctx, cen=cen,
                          uniq=uniq_nodes, colidx=colidx, W=W_t))

    # width-sort tiles per core; chunk widths shared across cores
    for co in cores:
        co["tperm"] = np.argsort(-co["W"], kind="stable")
    widths = []
    for ci in range(NCH):
        w = max(int(co["W"][co["tperm"][ci * CH]]) for co in cores)
        w = ((w + 15) // 16) * 16
        assert w <= SLOT, f"tile width {w} exceeds psum slot {SLOT}"
        widths.append(w)
    widths = tuple(widths)

    in_maps = []
    metas = []
    ident = np.eye(128, dtype=np_f8)
    s8_cols = 2 * sum(CH * w for w in widths)
    for co in cores:
        tperm = co["tperm"]
        el = (tperm[:, None] * 128 + np.arange(128)[None, :]).ravel()
        ctx_s = co["ctx"][el]
        cen_s = co["cen"][el]

        erows = embeddings[cen_s].astype(np.float16)           # [BC, D]
        embT = np.ascontiguousarray(erows.T)                   # [128, BC]
        emb_dve = np.ascontiguousarray(
            erows.reshape(T, 128, D).transpose(1, 0, 2).reshape(128, T * D))

        gs = []
        for l in L_DENSE:
            node = ctx_s >> l
            sgn = np.where(node % 2 == 0, -1.0, 1.0).astype(np.float32)
            rows = probs[node] * sgn[:, None]                  # -sign * P
            gs.append(np.ascontiguousarray(
                rows.astype(np.float16).reshape(T, 128, D)
                .transpose(1, 0, 2).reshape(128, T * D)))

        # fp16 slabs: embT alone (deep-critical), then [emb | g1 | g2]
        slab16 = np.empty((128, NCH * 3 * CH * D), dtype=np.float16)
        for c in range(NCH):
            base = c * 3 * CH * D
            cd = slice(c * CH * D, (c + 1) * CH * D)
            slab16[:, base: base + CH * D] = emb_dve[:, cd]
            slab16[:, base + CH * D: base + 2 * CH * D] = gs[0][:, cd]
            slab16[:, base + 2 * CH * D: base + 3 * CH * D] = gs[1][:, cd]

        # fp8 slab: per chunk [mov (packed W) | mask (packed W)]
        slab8 = np.zeros((128, s8_cols), dtype=np_f8)
        off = 0
        for c in range(NCH):
            W = widths[c]
            movb = np.zeros((128, CH * W), dtype=np_f8)
            maskb = np.empty((128, CH * W), dtype=np.float32)
            for i in range(CH):
                t = tperm[c * CH + i]
                u = co["uniq"][t]
                w_t = len(u)
                sgn = np.where(u % 2 == 0, -1.0, 1.0).astype(np.float32)
                blk = (probs[u] * sgn[:, None]).astype(np_f8)  # [w_t, 128]
                movb[:, i * W: i * W + w_t] = blk.T
                m = np.full((128, W), MASK_OFF, dtype=np.float32)
                m[np.arange(128)[:, None], co["colidx"][t]] = 0.0
                maskb[:, i * W: (i + 1) * W] = m
            slab8[:, off: off + CH * W] = movb
            slab8[:, off + CH * W: off + 2 * CH * W] = maskb.astype(np_f8)
            off += 2 * CH * W

        in_maps.append({
            "slabT": embT, "slab16": slab16, "slab8": slab8, "ident": ident,
        })
        metas.append(dict(shard=co["shard"], tperm=tperm))

    return widths, in_maps, metas


def kernel(center, context, embeddings, probs_tensor):
    import os
    from concourse.bass_utils import run_bass_kernel_spmd

    center = np.asarray(center)
    context = np.asarray(context)
    embeddings = np.asarray(embeddings, dtype=np.float32)
    probs = np.asarray(probs_tensor, dtype=np.float32)

    widths, in_maps, metas = _prep(center, context, embeddings, probs)

    key = ("nc", widths)
    if key not in _CACHE:
        _CACHE[key] = _build_program(widths)
    nc = _CACHE[key]

    res = run_bass_kernel_spmd(
        nc, in_maps, core_ids=list(range(N_CORES)),
        trace=os.environ.get("KERNEL_TRACE") == "1",
    )
    _CACHE["last_res"] = res

    loss = np.zeros(B_TOTAL, dtype=np.float32)
    for c in range(N_CORES):
        o = res.results[c]["out"]                  # [128, T] sorted-tile order
        m = metas[c]
        el = (m["tperm"][:, None] * 128 + np.arange(128)[None, :]).ravel()
        loss[m["shard"][el]] = o.T.ravel()
    return loss[:, None].astype(np.float32)


# revision 24
# speedup vs baseline: 1.0425x; 1.0425x over previous
"""DeepWalk hierarchical-softmax loss on 8 Trainium2 NeuronCores.

loss[b] = sum_{l=1..18} softplus(-sign_l * dot(probs[(ctx_b+N) >> l], emb[center_b]))

Strategy (v3 — per-chunk pipeline, balanced engines):
  Elements are context-sorted and split 8192/core (64 tiles of 128).
  Sorted order makes consecutive elements share tree nodes at levels >= 3:
  a 128-element tile touches only ~214 distinct (level, node) rows across
  levels 3..18 (vs 16*128 naive).

  Levels 3..18 ("PE path"): per tile, the distinct sign-folded rows are a
  [128d x W] fp8 moving block; stationary is the tile's embeddings
  transposed [128d x 128b] fp16.  One matmul gives every element's dot
  against every candidate row (PSUM [128b x W]).  Four 512-wide matmuls
  (identity stationary x fp8 bias matrix) add 0 to each element's OWN
  (level, node) columns and -64 to the rest; softplus(dot - 64) ~= 0, so
  a plain per-tile row sum after softplus performs the per-element
  selection with no gather/select ops at all.

  softplus = Ln(Exp(x)+1) on the scalar engine; an explicit
  InstLoadActFuncSet pins the combined natural_log_exp table once so the
  per-chunk Exp/Ln interleave never reloads tables (the v2 kernel lost
  18 us to table thrash).  The row sum runs as: two folds on the (idle)
  GPSIMD engine, then one segmented pool_avg on the DVE; the avg's 1/(W/4)
  factor is undone by a per-tile weight vector at the end.

  Levels 1..2 ("dense path"): nearly every element has a distinct node, so
  the rows are host-gathered per element (contiguous fp16 streams) and the
  dots run on the DVE as one multiply + two folds + one segmented pool_avg
  per chunk; the 1/32 avg factor is undone by the Exp activation's scale.

  All six per-chunk input tensors are packed into two DMA slabs (one fp16,
  one fp8) so the stream is 2 triggers/chunk and arrives in issue order.
  Tiles are width-sorted per core so each 8-tile chunk's padded width
  tracks the envelope, not the global max; the last chunk (narrowest) also
  minimizes the pipeline tail.  All host work is layout only: sorting,
  index building, row gathering/sign folding, dtype casts.
"""

import numpy as np

N_NODES = 524288          # 2**19
D = 128
B_TOTAL = 65536
N_CORES = 8
BC = B_TOTAL // N_CORES   # 8192
T = BC // 128             # 64 tiles/core
CH = 8                    # tiles per chunk
NCH = T // CH             # 8 chunks
SLOT = 256                # psum slot stride (fp32 elems); 2 slots per bank
L_DENSE = (1, 2)
L_PE = tuple(range(3, 19))
MASK_OFF = -64.0          # softplus(dot + MASK_OFF) ~= 0 for |dot| <~ 40

_CACHE = {}


def _build_program(widths):
    """widths: per-chunk padded column counts (shared across cores)."""
    import os as _os
    import concourse.mybir as mybir
    import concourse.tile as tile
    import concourse.bacc as bacc
    from concourse.hw_specs import get_activation_tables

    f32 = mybir.dt.float32
    f16 = mybir.dt.float16
    f8 = mybir.dt.float8e4
    mult = mybir.AluOpType.mult
    add = mybir.AluOpType.add
    f_exp = mybir.ActivationFunctionType.Exp
    f_ln = mybir.ActivationFunctionType.Ln

    nc = bacc.Bacc("TRN2", target_bir_lowering=False, debug=False,
                   num_devices=int(_os.environ.get("KERNEL_NDEV", N_CORES)))

    tab_names = list(get_activation_tables(nc.m.arch).keys())
    LN_EXP_TABLE = tab_names.index("natural_log_exp_and_others")

    mv_cols = sum(CH * w for w in widths)          # mov cols (packed W)
    s8_cols = 2 * mv_cols                          # + mask cols (packed W)
    slabT = nc.dram_tensor("slabT", (128, NCH * CH * D), f16,
                           kind="ExternalInput").ap()
    slab16 = nc.dram_tensor("slab16", (128, NCH * 3 * CH * D), f16,
                            kind="ExternalInput").ap()
    slab8 = nc.dram_tensor("slab8", (128, s8_cols), f8,
                           kind="ExternalInput").ap()
    ident = nc.dram_tensor("ident", (128, 128), f8, kind="ExternalInput").ap()
    out = nc.dram_tensor("out", (128, T), f32, kind="ExternalOutput").ap()

    with tile.TileContext(nc) as tc:
        with tc.tile_pool(name="cst", bufs=1) as pcst, \
             tc.tile_pool(name="sT", bufs=6) as psT, \
             tc.tile_pool(name="s16", bufs=6) as ps16, \
             tc.tile_pool(name="s8", bufs=6) as ps8, \
             tc.tile_pool(name="sp", bufs=2) as psp, \
             tc.tile_pool(name="df", bufs=2) as pdf, \
             tc.tile_pool(name="dn", bufs=2) as pdn, \
             tc.tile_pool(name="acc", bufs=1) as pacc, \
             tc.tile_pool(name="ps", bufs=2, space="PSUM") as pps:

            # pin the combined exp+ln activation table once
            nc.scalar.add_instruction(mybir.InstLoadActFuncSet(
                name=nc.get_next_instruction_name(), ins=[], outs=[],
                act_func_set_id=LN_EXP_TABLE))

            t_id = pcst.tile([128, 128], f8, tag="ID")
            nc.gpsimd.dma_start(out=t_id[:], in_=ident)

            ydeep = pacc.tile([128, T], f32, tag="YD")    # pooled deep avgs
            y12 = pacc.tile([128, 2 * T], f32, tag="Y12")  # pooled dense avgs

            off8 = 0
            for c in range(NCH):
                W = widths[c]
                H, Q = W // 2, W // 4
                c16 = slice(c * 3 * CH * D, (c + 1) * 3 * CH * D)
                cT = slice(c * CH * D, (c + 1) * CH * D)
                tT = psT.tile([128, CH * D], f16, tag="ST")
                t16 = ps16.tile([128, 3 * CH * D], f16, tag="S16")
                t8 = ps8.tile([128, 2 * CH * W], f8, tag="S8")
                # deep-path (ACT cadence) operands stream first
                nc.sync.dma_start(out=tT[:], in_=slabT[:, cT])
                nc.sync.dma_start(
                    out=t8[:], in_=slab8[:, off8: off8 + 2 * CH * W])
                nc.sync.dma_start(out=t16[:], in_=slab16[:, c16])
                off8 += 2 * CH * W

                embT = tT[:, 0: CH * D]
                emb = t16[:, 0: CH * D]
                g1 = t16[:, CH * D: 2 * CH * D]
                g2 = t16[:, 2 * CH * D: 3 * CH * D]
                mov = t8[:, 0: CH * W]
                mask = t8[:, CH * W: 2 * CH * W]

                # ---- PE path: dots + bias for CH tiles into one psum ----
                # start=True clears the whole 2-slot PSUM bank, so only the
                # even-slot matmul per bank sets it; 512-wide identity
                # matmuls then add the per-element -64/0 bias per bank.
                big = pps.tile([128, CH * SLOT], f32, tag="PS")
                for i in range(0, CH, 2):
                    nc.tensor.matmul(
                        big[:, i * SLOT: i * SLOT + W],
                        lhsT=embT[:, i * 128: (i + 1) * 128],
                        rhs=mov[:, i * W: (i + 1) * W],
                        start=True, stop=False, skip_group_check=True)
                for i in range(1, CH, 2):
                    nc.tensor.matmul(
                        big[:, i * SLOT: i * SLOT + W],
                        lhsT=embT[:, i * 128: (i + 1) * 128],
                        rhs=mov[:, i * W: (i + 1) * W],
                        start=False, stop=False, skip_group_check=True)
                for i in range(CH):
                    nc.tensor.matmul(
                        big[:, i * SLOT: i * SLOT + W],
                        lhsT=t_id[:],
                        rhs=mask[:, i * W: (i + 1) * W],
                        start=False, stop=True, skip_group_check=True)

                # ---- softplus = Ln(Exp(x) + 1) ----
                # Exp runs in place in PSUM (elementwise streaming RMW), so
                # no fp32 intermediate ever touches SBUF; Ln then writes the
                # fp16 softplus straight to SBUF.
                ps3 = big[:].rearrange("p (s w) -> p s w", w=SLOT)[:, :, :W]
                nc.scalar.activation(out=ps3, in_=ps3, func=f_exp)
                t_sp = psp.tile([128, CH * W], f16, tag="SP")
                sp3 = t_sp[:].rearrange("p (s w) -> p s w", w=W)
                nc.scalar.activation(out=sp3, in_=ps3, func=f_ln, bias=1.0)

                # ---- deep row-sum: 2 folds on GPSIMD + pool_avg on DVE ----
                t_f1 = pdf.tile([128, CH * H], f16, tag="DF1")
                f13 = t_f1[:].rearrange("p (s w) -> p s w", w=H)
                nc.gpsimd.tensor_tensor(
                    out=f13, in0=sp3[:, :, 0:H], in1=sp3[:, :, H:W], op=add)
                t_f2 = pdf.tile([128, CH * Q], f16, tag="DF2")
                f23 = t_f2[:].rearrange("p (s w) -> p s w", w=Q)
                nc.gpsimd.tensor_tensor(
                    out=f23, in0=f13[:, :, 0:Q], in1=f13[:, :, Q:H], op=add)

                # ---- dense path: levels 1..2 on DVE ----
                prod = pdn.tile([128, 2 * CH * D], f16, tag="PR")
                nc.vector.tensor_tensor(
                    out=prod[:, 0: CH * D], in0=g1, in1=emb, op=mult)
                nc.vector.tensor_tensor(
                    out=prod[:, CH * D: 2 * CH * D], in0=g2, in1=emb, op=mult)
                p4 = prod[:].rearrange("p (l s d) -> p l s d", l=2, d=D)
                t_p1 = pdn.tile([128, 2 * CH * (D // 2)], f16, tag="PF1")
                p14 = t_p1[:].rearrange("p (l s d) -> p l s d", l=2, d=D // 2)
                nc.vector.tensor_tensor(
                    out=p14, in0=p4[:, :, :, 0: D // 2],
                    in1=p4[:, :, :, D // 2: D], op=add)
                t_p2 = pdn.tile([128, 2 * CH * (D // 4)], f16, tag="PF2")
                p24 = t_p2[:].rearrange("p (l s d) -> p l s d", l=2, d=D // 4)
                nc.vector.tensor_tensor(
                    out=p24, in0=p14[:, :, :, 0: D // 4],
                    in1=p14[:, :, :, D // 4: D // 2], op=add)
                y124 = y12[:].rearrange("p (l t) -> p l t", l=2)
                nc.vector.tensor_reduce(
                    out=y124[:, :, c * CH: (c + 1) * CH], in_=p24,
                    axis=mybir.AxisListType.X, op=add)
                nc.vector.tensor_reduce(
                    out=ydeep[:, c * CH: (c + 1) * CH], in_=f23,
                    axis=mybir.AxisListType.X, op=add)

            # ---- tail: softplus(dense dots) and final sum ----
            sp12 = pacc.tile([128, 2 * T], f32, tag="SP12")
            nc.scalar.activation(out=sp12[:], in_=y12[:], func=f_exp)
            nc.scalar.activation(out=sp12[:], in_=sp12[:], func=f_ln, bias=1.0)
            tot = pacc.tile([128, T], f32, tag="TOT")
            nc.vector.tensor_tensor(
                out=tot[:], in0=ydeep[:], in1=sp12[:, 0: T], op=add)
            nc.vector.tensor_tensor(
                out=tot[:], in0=tot[:], in1=sp12[:, T: 2 * T], op=add)
            nc.sync.dma_start(out=out, in_=tot[:])

    nc.compile()
    return nc


def _prep(center, context, embeddings, probs):
    """Host layout prep: sort, shard, build per-core tensors."""
    import concourse.mybir as mybir
    np_f8 = mybir.dt.np(mybir.dt.float8e4)

    perm = np.argsort(context, kind="stable")

    cores = []
    for c in range(N_CORES):
        shard = perm[c * BC: (c + 1) * BC]
        ctx = context[shard].astype(np.int64) + N_NODES
        cen = center[shard].astype(np.int64)

        # per-tile distinct rows for levels 3..18
        uniq_nodes = []   # per tile: concat distinct node ids (level-major)
        colidx = np.empty((T, 128, len(L_PE)), dtype=np.int64)
        W_t = np.empty(T, dtype=np.int64)
        for t in range(T):
            tc = ctx[t * 128: (t + 1) * 128]
            nodes_t = []
            base = 0
            for k, l in enumerate(L_PE):
                u, inv = np.unique(tc >> l, return_inverse=True)
                nodes_t.append(u)
                colidx[t, :, k] = base + inv
                base += len(u)
            uniq_nodes.append(np.concatenate(nodes_t))
            W_t[t] = base

        cores.append(dict(shard=shard, ctx=ctx, cen=cen,
                          uniq=uniq_nodes, colidx=colidx, W=W_t))

    # width-sort tiles per core; chunk widths shared across cores
    for co in cores:
        co["tperm"] = np.argsort(-co["W"], kind="stable")
    widths = []
    for ci in range(NCH):
        w = max(int(co["W"][co["tperm"][ci * CH]]) for co in cores)
        w = ((w + 15) // 16) * 16
        assert w <= SLOT, f"tile width {w} exceeds psum slot {SLOT}"
        widths.append(w)
    widths = tuple(widths)

    in_maps = []
    metas = []
    ident = np.eye(128, dtype=np_f8)
    s8_cols = 2 * sum(CH * w for w in widths)
    for co in cores:
        tperm = co["tperm"]
        el = (tperm[:, None] * 128 + np.arange(128)[None, :]).ravel()
        ctx_s = co["ctx"][el]
        cen_s = co["cen"][el]

        erows = embeddings[cen_s].astype(np.float16)           # [BC, D]
        embT = np.ascontiguousarray(erows.T)                   # [128, BC]
        emb_dve = np.ascontiguousarray(
            erows.reshape(T, 128, D).transpose(1, 0, 2).reshape(128, T * D))

        gs = []
        for l in L_DENSE:
            node = ctx_s >> l
            sgn = np.where(node % 2 == 0, -1.0, 1.0).astype(np.float32)
            rows = probs[node] * sgn[:, None]                  # -sign * P
            gs.append(np.ascontiguousarray(
                rows.astype(np.float16).reshape(T, 128, D)
                .transpose(1, 0, 2).reshape(128, T * D)))

        # fp16 slabs: embT alone (deep-critical), then [emb | g1 | g2]
        slab16 = np.empty((128, NCH * 3 * CH * D), dtype=np.float16)
        for c in range(NCH):
            base = c * 3 * CH * D
            cd = slice(c * CH * D, (c + 1) * CH * D)
            slab16[:, base: base + CH * D] = emb_dve[:, cd]
            slab16[:, base + CH * D: base + 2 * CH * D] = gs[0][:, cd]
            slab16[:, base + 2 * CH * D: base + 3 * CH * D] = gs[1][:, cd]

        # fp8 slab: per chunk [mov (packed W) | mask (packed W)]
        slab8 = np.zeros((128, s8_cols), dtype=np_f8)
        off = 0
        for c in range(NCH):
            W = widths[c]
            movb = np.zeros((128, CH * W), dtype=np_f8)
            maskb = np.empty((128, CH * W), dtype=np.float32)
            for i in range(CH):
                t = tperm[c * CH + i]
                u = co["uniq"][t]
                w_t = len(u)
                sgn = np.where(u % 2 == 0, -1.0, 1.0).astype(np.float32)
                blk = (probs[u] * sgn[:, None]).astype(np_f8)  # [w_t, 128]
                movb[:, i * W: i * W + w_t] = blk.T
                m = np.full((128, W), MASK_OFF, dtype=np.float32)
                m[np.arange(128)[:, None], co["colidx"][t]] = 0.0
                maskb[:, i * W: (i + 1) * W] = m
            slab8[:, off: off + CH * W] = movb
            slab8[:, off + CH * W: off + 2 * CH * W] = maskb.astype(np_f8)
            off += 2 * CH * W

        in_maps.append({
            "slabT": embT, "slab16": slab16, "slab8": slab8, "ident": ident,
        })
        metas.append(dict(shard=co["shard"], tperm=tperm))

    return widths, in_maps, metas


def kernel(center, context, embeddings, probs_tensor):
    import os
    from concourse.bass_utils import run_bass_kernel_spmd

    center = np.asarray(center)
    context = np.asarray(context)
    embeddings = np.asarray(embeddings, dtype=np.float32)
    probs = np.asarray(probs_tensor, dtype=np.float32)

    widths, in_maps, metas = _prep(center, context, embeddings, probs)

    key = ("nc", widths)
    if key not in _CACHE:
        _CACHE[key] = _build_program(widths)
    nc = _CACHE[key]

    res = run_bass_kernel_spmd(
        nc, in_maps, core_ids=list(range(N_CORES)),
        trace=os.environ.get("KERNEL_TRACE") == "1",
    )
    _CACHE["last_res"] = res

    loss = np.zeros(B_TOTAL, dtype=np.float32)
    for c in range(N_CORES):
        o = res.results[c]["out"]                  # [128, T] sorted-tile order
        m = metas[c]
        el = (m["tperm"][:, None] * 128 + np.arange(128)[None, :]).ravel()
        loss[m["shard"][el]] = o.T.ravel()
    return loss[:, None].astype(np.float32)


# revision 25
# speedup vs baseline: 1.1948x; 1.1461x over previous
"""DeepWalk hierarchical-softmax loss on 8 Trainium2 NeuronCores.

loss[b] = sum_{l=1..18} softplus(-sign_l * dot(probs[(ctx_b+N) >> l], emb[center_b]))

Strategy (v3 — per-chunk pipeline, balanced engines):
  Elements are context-sorted and split 8192/core (64 tiles of 128).
  Sorted order makes consecutive elements share tree nodes at levels >= 3:
  a 128-element tile touches only ~214 distinct (level, node) rows across
  levels 3..18 (vs 16*128 naive).

  Levels 3..18 ("PE path"): per tile, the distinct sign-folded rows are a
  [128d x W] fp8 moving block; stationary is the tile's embeddings
  transposed [128d x 128b] fp16.  One matmul gives every element's dot
  against every candidate row (PSUM [128b x W]).  Four 512-wide matmuls
  (identity stationary x fp8 bias matrix) add 0 to each element's OWN
  (level, node) columns and -64 to the rest; softplus(dot - 64) ~= 0, so
  a plain per-tile row sum after softplus performs the per-element
  selection with no gather/select ops at all.

  softplus = Ln(Exp(x)+1) on the scalar engine; an explicit
  InstLoadActFuncSet pins the combined natural_log_exp table once so the
  per-chunk Exp/Ln interleave never reloads tables (the v2 kernel lost
  18 us to table thrash).  The row sum runs as: two folds on the (idle)
  GPSIMD engine, then one segmented pool_avg on the DVE; the avg's 1/(W/4)
  factor is undone by a per-tile weight vector at the end.

  Levels 1..2 ("dense path"): nearly every element has a distinct node, so
  the rows are host-gathered per element (contiguous fp16 streams) and the
  dots run on the DVE as one multiply + two folds + one segmented pool_avg
  per chunk; the 1/32 avg factor is undone by the Exp activation's scale.

  All six per-chunk input tensors are packed into two DMA slabs (one fp16,
  one fp8) so the stream is 2 triggers/chunk and arrives in issue order.
  Tiles are width-sorted per core so each 8-tile chunk's padded width
  tracks the envelope, not the global max; the last chunk (narrowest) also
  minimizes the pipeline tail.  All host work is layout only: sorting,
  index building, row gathering/sign folding, dtype casts.
"""

import numpy as np

N_NODES = 524288          # 2**19
D = 128
B_TOTAL = 65536
N_CORES = 8
BC = B_TOTAL // N_CORES   # 8192
T = BC // 128             # 64 tiles/core
CH = 8                    # tiles per chunk
NCH = T // CH             # 8 chunks
SLOT = 256                # psum slot stride (fp32 elems); 2 slots per bank
L_DENSE = (1, 2)
L_PE = tuple(range(3, 19))
MASK_OFF = -64.0          # softplus(dot + MASK_OFF) ~= 0 for |dot| <~ 40

_CACHE = {}


def _build_program(widths):
    """widths: per-chunk padded column counts (shared across cores)."""
    import os as _os
    import concourse.mybir as mybir
    import concourse.tile as tile
    import concourse.bacc as bacc
    from concourse.hw_specs import get_activation_tables

    f32 = mybir.dt.float32
    f16 = mybir.dt.float16
    f8 = mybir.dt.float8e4
    mult = mybir.AluOpType.mult
    add = mybir.AluOpType.add
    f_exp = mybir.ActivationFunctionType.Exp
    f_ln = mybir.ActivationFunctionType.Ln

    nc = bacc.Bacc("TRN2", target_bir_lowering=False, debug=False,
                   num_devices=int(_os.environ.get("KERNEL_NDEV", N_CORES)))

    tab_names = list(get_activation_tables(nc.m.arch).keys())
    LN_EXP_TABLE = tab_names.index("natural_log_exp_and_others")

    mv_cols = sum(CH * w for w in widths)          # mov cols (packed W)
    s8_cols = 2 * mv_cols                          # + mask cols (packed W)
    slabT = nc.dram_tensor("slabT", (128, NCH * CH * D), f16,
                           kind="ExternalInput").ap()
    slab16 = nc.dram_tensor("slab16", (128, NCH * 3 * CH * D), f16,
                            kind="ExternalInput").ap()
    slab8 = nc.dram_tensor("slab8", (128, s8_cols), f8,
                           kind="ExternalInput").ap()
    ident = nc.dram_tensor("ident", (128, 128), f8, kind="ExternalInput").ap()
    out = nc.dram_tensor("out", (128, T), f32, kind="ExternalOutput").ap()

    with tile.TileContext(nc) as tc:
        with tc.tile_pool(name="cst", bufs=1) as pcst, \
             tc.tile_pool(name="sT", bufs=6) as psT, \
             tc.tile_pool(name="s16", bufs=6) as ps16, \
             tc.tile_pool(name="s8", bufs=6) as ps8, \
             tc.tile_pool(name="sp", bufs=2) as psp, \
             tc.tile_pool(name="df", bufs=2) as pdf, \
             tc.tile_pool(name="dn", bufs=2) as pdn, \
             tc.tile_pool(name="acc", bufs=1) as pacc, \
             tc.tile_pool(name="ps", bufs=2, space="PSUM") as pps:

            # pin the combined exp+ln activation table once
            nc.scalar.add_instruction(mybir.InstLoadActFuncSet(
                name=nc.get_next_instruction_name(), ins=[], outs=[],
                act_func_set_id=LN_EXP_TABLE))

            t_id = pcst.tile([128, 128], f8, tag="ID")
            nc.gpsimd.dma_start(out=t_id[:], in_=ident)

            ydeep = pacc.tile([128, T], f32, tag="YD")    # pooled deep avgs
            y12 = pacc.tile([128, 2 * T], f32, tag="Y12")  # pooled dense avgs

            off8 = 0
            for c in range(NCH):
                W = widths[c]
                H, Q = W // 2, W // 4
                c16 = slice(c * 3 * CH * D, (c + 1) * 3 * CH * D)
                cT = slice(c * CH * D, (c + 1) * CH * D)
                tT = psT.tile([128, CH * D], f16, tag="ST")
                t16 = ps16.tile([128, 3 * CH * D], f16, tag="S16")
                t8 = ps8.tile([128, 2 * CH * W], f8, tag="S8")
                # deep-path (ACT cadence) operands stream first
                nc.sync.dma_start(out=tT[:], in_=slabT[:, cT])
                nc.sync.dma_start(
                    out=t8[:], in_=slab8[:, off8: off8 + 2 * CH * W])
                nc.sync.dma_start(out=t16[:], in_=slab16[:, c16])
                off8 += 2 * CH * W

                embT = tT[:, 0: CH * D]
                emb = t16[:, 0: CH * D]
                g1 = t16[:, CH * D: 2 * CH * D]
                g2 = t16[:, 2 * CH * D: 3 * CH * D]
                mov = t8[:, 0: CH * W]
                mask = t8[:, CH * W: 2 * CH * W]

                # ---- PE path: dots + bias for CH tiles into one psum ----
                # start=True clears the whole 2-slot PSUM bank, so only the
                # even-slot matmul per bank sets it; 512-wide identity
                # matmuls then add the per-element -64/0 bias per bank.
                big = pps.tile([128, CH * SLOT], f32, tag="PS")
                for i in range(0, CH, 2):
                    nc.tensor.matmul(
                        big[:, i * SLOT: i * SLOT + W],
                        lhsT=embT[:, i * 128: (i + 1) * 128],
                        rhs=mov[:, i * W: (i + 1) * W],
                        start=True, stop=False, skip_group_check=True)
                for i in range(1, CH, 2):
                    nc.tensor.matmul(
                        big[:, i * SLOT: i * SLOT + W],
                        lhsT=embT[:, i * 128: (i + 1) * 128],
                        rhs=mov[:, i * W: (i + 1) * W],
                        start=False, stop=False, skip_group_check=True)
                for i in range(CH):
                    nc.tensor.matmul(
                        big[:, i * SLOT: i * SLOT + W],
                        lhsT=t_id[:],
                        rhs=mask[:, i * W: (i + 1) * W],
                        start=False, stop=True, skip_group_check=True)

                # ---- softplus = Ln(Exp(x) + 1) ----
                # Exp runs in place in PSUM (elementwise streaming RMW), so
                # no fp32 intermediate ever touches SBUF; Ln then writes the
                # fp16 softplus straight to SBUF.
                ps3 = big[:].rearrange("p (s w) -> p s w", w=SLOT)[:, :, :W]
                nc.scalar.activation(out=ps3, in_=ps3, func=f_exp)
                t_sp = psp.tile([128, CH * W], f16, tag="SP")
                sp3 = t_sp[:].rearrange("p (s w) -> p s w", w=W)
                nc.scalar.activation(out=sp3, in_=ps3, func=f_ln, bias=1.0)

                # ---- deep row-sum: 2 folds on GPSIMD + pool_avg on DVE ----
                t_f1 = pdf.tile([128, CH * H], f16, tag="DF1")
                f13 = t_f1[:].rearrange("p (s w) -> p s w", w=H)
                nc.vector.tensor_tensor(
                    out=f13, in0=sp3[:, :, 0:H], in1=sp3[:, :, H:W], op=add)
                t_f2 = pdf.tile([128, CH * Q], f16, tag="DF2")
                f23 = t_f2[:].rearrange("p (s w) -> p s w", w=Q)
                nc.vector.tensor_tensor(
                    out=f23, in0=f13[:, :, 0:Q], in1=f13[:, :, Q:H], op=add)

                # ---- dense path: levels 1..2 on DVE ----
                prod = pdn.tile([128, 2 * CH * D], f16, tag="PR")
                nc.vector.tensor_tensor(
                    out=prod[:, 0: CH * D], in0=g1, in1=emb, op=mult)
                nc.vector.tensor_tensor(
                    out=prod[:, CH * D: 2 * CH * D], in0=g2, in1=emb, op=mult)
                p4 = prod[:].rearrange("p (l s d) -> p l s d", l=2, d=D)
                t_p1 = pdn.tile([128, 2 * CH * (D // 2)], f16, tag="PF1")
                p14 = t_p1[:].rearrange("p (l s d) -> p l s d", l=2, d=D // 2)
                nc.vector.tensor_tensor(
                    out=p14, in0=p4[:, :, :, 0: D // 2],
                    in1=p4[:, :, :, D // 2: D], op=add)
                t_p2 = pdn.tile([128, 2 * CH * (D // 4)], f16, tag="PF2")
                p24 = t_p2[:].rearrange("p (l s d) -> p l s d", l=2, d=D // 4)
                nc.vector.tensor_tensor(
                    out=p24, in0=p14[:, :, :, 0: D // 4],
                    in1=p14[:, :, :, D // 4: D // 2], op=add)
                y124 = y12[:].rearrange("p (l t) -> p l t", l=2)
                nc.vector.tensor_reduce(
                    out=y124[:, :, c * CH: (c + 1) * CH], in_=p24,
                    axis=mybir.AxisListType.X, op=add)
                nc.vector.tensor_reduce(
                    out=ydeep[:, c * CH: (c + 1) * CH], in_=f23,
                    axis=mybir.AxisListType.X, op=add)

            # ---- tail: softplus(dense dots) and final sum ----
            sp12 = pacc.tile([128, 2 * T], f32, tag="SP12")
            nc.scalar.activation(out=sp12[:], in_=y12[:], func=f_exp)
            nc.scalar.activation(out=sp12[:], in_=sp12[:], func=f_ln, bias=1.0)
            tot = pacc.tile([128, T], f32, tag="TOT")
            nc.vector.tensor_tensor(
                out=tot[:], in0=ydeep[:], in1=sp12[:, 0: T], op=add)
            nc.vector.tensor_tensor(
                out=tot[:], in0=tot[:], in1=sp12[:, T: 2 * T], op=add)
            nc.sync.dma_start(out=out, in_=tot[:])

    nc.compile()
    return nc


def _prep(center, context, embeddings, probs):
    """Host layout prep: sort, shard, build per-core tensors."""
    import concourse.mybir as mybir
    np_f8 = mybir.dt.np(mybir.dt.float8e4)

    perm = np.argsort(context, kind="stable")

    cores = []
    for c in range(N_CORES):
        shard = perm[c * BC: (c + 1) * BC]
        ctx = context[shard].astype(np.int64) + N_NODES
        cen = center[shard].astype(np.int64)

        # per-tile distinct rows for levels 3..18
        uniq_nodes = []   # per tile: concat distinct node ids (level-major)
        colidx = np.empty((T, 128, len(L_PE)), dtype=np.int64)
        W_t = np.empty(T, dtype=np.int64)
        for t in range(T):
            tc = ctx[t * 128: (t + 1) * 128]
            nodes_t = []
            base = 0
            for k, l in enumerate(L_PE):
                u, inv = np.unique(tc >> l, return_inverse=True)
                nodes_t.append(u)
                colidx[t, :, k] = base + inv
                base += len(u)
            uniq_nodes.append(np.concatenate(nodes_t))
            W_t[t] = base

        cores.append(dict(shard=shard, ctx=ctx, cen=cen,
                          uniq=uniq_nodes, colidx=colidx, W=W_t))

    # width-sort tiles per core; chunk widths shared across cores
    for co in cores:
        co["tperm"] = np.argsort(-co["W"], kind="stable")
    widths = []
    for ci in range(NCH):
        w = max(int(co["W"][co["tperm"][ci * CH]]) for co in cores)
        w = ((w + 15) // 16) * 16
        assert w <= SLOT, f"tile width {w} exceeds psum slot {SLOT}"
        widths.append(w)
    widths = tuple(widths)

    in_maps = []
    metas = []
    ident = np.eye(128, dtype=np_f8)
    s8_cols = 2 * sum(CH * w for w in widths)
    for co in cores:
        tperm = co["tperm"]
        el = (tperm[:, None] * 128 + np.arange(128)[None, :]).ravel()
        ctx_s = co["ctx"][el]
        cen_s = co["cen"][el]

        erows = embeddings[cen_s].astype(np.float16)           # [BC, D]
        embT = np.ascontiguousarray(erows.T)                   # [128, BC]
        emb_dve = np.ascontiguousarray(
            erows.reshape(T, 128, D).transpose(1, 0, 2).reshape(128, T * D))

        gs = []
        for l in L_DENSE:
            node = ctx_s >> l
            sgn = np.where(node % 2 == 0, -1.0, 1.0).astype(np.float32)
            rows = probs[node] * sgn[:, None]                  # -sign * P
            gs.append(np.ascontiguousarray(
                rows.astype(np.float16).reshape(T, 128, D)
                .transpose(1, 0, 2).reshape(128, T * D)))

        # fp16 slabs: embT alone (deep-critical), then [emb | g1 | g2]
        slab16 = np.empty((128, NCH * 3 * CH * D), dtype=np.float16)
        for c in range(NCH):
            base = c * 3 * CH * D
            cd = slice(c * CH * D, (c + 1) * CH * D)
            slab16[:, base: base + CH * D] = emb_dve[:, cd]
            slab16[:, base + CH * D: base + 2 * CH * D] = gs[0][:, cd]
            slab16[:, base + 2 * CH * D: base + 3 * CH * D] = gs[1][:, cd]

        # fp8 slab: per chunk [mov (packed W) | mask (packed W)]
        slab8 = np.zeros((128, s8_cols), dtype=np_f8)
        off = 0
        for c in range(NCH):
            W = widths[c]
            movb = np.zeros((128, CH * W), dtype=np_f8)
            maskb = np.empty((128, CH * W), dtype=np.float32)
            for i in range(CH):
                t = tperm[c * CH + i]
                u = co["uniq"][t]
                w_t = len(u)
                sgn = np.where(u % 2 == 0, -1.0, 1.0).astype(np.float32)
                blk = (probs[u] * sgn[:, None]).astype(np_f8)  # [w_t, 128]
                movb[:, i * W: i * W + w_t] = blk.T
                m = np.full((128, W), MASK_OFF, dtype=np.float32)
                m[np.arange(128)[:, None], co["colidx"][t]] = 0.0
                maskb[:, i * W: (i + 1) * W] = m
            slab8[:, off: off + CH * W] = movb
            slab8[:, off + CH * W: off + 2 * CH * W] = maskb.astype(np_f8)
            off += 2 * CH * W

        in_maps.append({
            "slabT": embT, "slab16": slab16, "slab8": slab8, "ident": ident,
        })
        metas.append(dict(shard=co["shard"], tperm=tperm))

    return widths, in_maps, metas


def kernel(center, context, embeddings, probs_tensor):
    import os
    from concourse.bass_utils import run_bass_kernel_spmd

    center = np.asarray(center)
    context = np.asarray(context)
    embeddings = np.asarray(embeddings, dtype=np.float32)
    probs = np.asarray(probs_tensor, dtype=np.float32)

    widths, in_maps, metas = _prep(center, context, embeddings, probs)

    key = ("nc", widths)
    if key not in _CACHE:
        _CACHE[key] = _build_program(widths)
    nc = _CACHE[key]

    res = run_bass_kernel_spmd(
        nc, in_maps, core_ids=list(range(N_CORES)),
        trace=os.environ.get("KERNEL_TRACE") == "1",
    )
    _CACHE["last_res"] = res

    loss = np.zeros(B_TOTAL, dtype=np.float32)
    for c in range(N_CORES):
        o = res.results[c]["out"]                  # [128, T] sorted-tile order
        m = metas[c]
        el = (m["tperm"][:, None] * 128 + np.arange(128)[None, :]).ravel()
        loss[m["shard"][el]] = o.T.ravel()
    return loss[:, None].astype(np.float32)
